# revision 2
# baseline (speedup 1.0000x reference)
"""Trainium2 Bass kernel for the 1D advection stencil (slope-limited flux).

Math (axis=-1, L = N + 4 ghost cells, th = 2.0):
    flux = rho * v
    d[i]  = flux[i+1] - flux[i]
    hs[i] = minmod3(d[i], (d[i]+d[i+1])/4, d[i+1])        # == 0.5*minmod3(c0,c1,c2)
    p[i]  = flux[i+1] - hs[i];  q[i] = flux[i+1] + hs[i]
    pm[i] = (v[i+1] < 0) * p[i];  qm[i] = (v[i+1] > 0) * q[i]
    pm[L-3] = 0; qm[0] = 0
    fn[j]  = pm[j+1] + qm[j]
    out[i] = fn[i] - fn[i+1]
minmod3(a,b,c) = max(min3, min(max3, 0)) -- selects min-|.| when all same
sign else 0.

Implementation notes (measured/derived on this container):
  * DVE rates: tensor_tensor fp32 = 1x, bf16 = 2x (2x_1p); tensor_scalar
    bf16 = 4x (4x_2p); scalar_tensor_tensor = 1x ALWAYS (no fast uops).
    So the kernel avoids STT entirely: bf16 TTs + bf16 TS ops.
  * rel-err budget is 2e-2; full bf16 pipeline measures 3.5e-3 vs the
    fp32 reference (validated in numpy with ml_dtypes).
  * ACT (scalar engine) cannot do 2-tensor ops (bias is per-partition
    scalar only) but converts dtypes at 1x @1.2GHz off the DVE critical
    path: rho/v fp32->bf16 up front, out bf16->fp32 at the end.
  * Custom fused DVE ops do NOT compile in this container (walrus
    codegen rejects InstCustomDveAnt: "ISA wrong length") -- verified
    against both a fresh op and production ops (AFFINE_THEN_ADD).
  * Pool/GpSimd shares an SBUF port with DVE (exclusive lock) and is
    2x slower -- net harmful, unused.

Sharding: pure data-parallel over the leading batch axis B=16 -> 2 slabs
per core on 8 cores.  No halo exchange needed (stencil couples only along
the last axis, which stays whole on every core).
"""

import numpy as np

import concourse.bass as bass
import concourse.mybir as mybir
from concourse.mybir import AluOpType
from concourse.tile import TileContext
from concourse.bass_utils import run_bass_kernel_spmd

# Problem shape (hardcoded; kernel.py must be self-contained).
B, M, L = 16, 256, 8192
NCORES = 8
BP = B // NCORES            # 2 batch slabs per core
ROWS = BP * M               # 512 rows per core
RT = ROWS // 128            # 4 partition tiles of 128 rows
OUT_L = L - 4               # 8188
CHUNK = 4094                # output columns per inner tile (2 chunks/row-tile)
F32 = mybir.dt.float32
BF16 = mybir.dt.bfloat16
COPY = mybir.ActivationFunctionType.Copy


def _split_multi_waits(nc):
    """Walrus in this environment rejects instructions carrying more than
    one sync wait ("Too many sync wait commands").  Tile freely attaches
    several.  Split: for an instruction with k>1 waits, emit k-1 engine
    NoOps (one wait each) immediately before it, leaving one wait on the
    instruction itself."""
    import copy
    import concourse.mybir as mybir

    counter = [0]

    def mk_nop(engine, wait):
        counter[0] += 1
        return mybir.InstNoOp(
            name=f"waitsplit-{counter[0]}",
            engine=engine,
            ins=[],
            outs=[],
            sync_info=mybir.SyncInfo(on_wait=[wait], on_update=[]),
        )

    m = nc.m
    new_module = copy.replace(m, functions=[])
    for function in m.functions:
        new_function = copy.replace(function, blocks=[])
        new_function.set_allocations_from_list(function.allocations)
        for block in function.blocks:
            new_insts = []
            for inst in block.instructions:
                si = inst.sync_info
                waits = list(si.on_wait) if (si and si.on_wait) else []
                if len(waits) > 1:
                    for w in waits[:-1]:
                        new_insts.append(mk_nop(inst.engine, w))
                    inst.sync_info = mybir.SyncInfo(
                        on_wait=[waits[-1]], on_update=list(si.on_update)
                    )
                new_insts.append(inst)
            new_function.blocks.append(
                copy.replace(block, instructions=new_insts)
            )
        new_module.functions.append(new_function)
    nc.m = new_module


def build_module(repeat=1, variant="bf16"):
    """repeat>1 wraps the whole body in a device-side For_i loop --
    benchmark-only (re-reads the same inputs, re-writes the same outputs)
    so device time dominates the axon tunnel overhead.

    variant: "bf16" (default; ACT-assisted bf16 pipeline),
             "dma" (transfers only -- roofline probe)."""
    import contextlib
    nc = bass.Bass()
    rho = nc.dram_tensor("rho", [ROWS, L], F32, kind="ExternalInput")
    vin = nc.dram_tensor("v", [ROWS, L], F32, kind="ExternalInput")
    out = nc.dram_tensor("out", [ROWS, OUT_L], F32, kind="ExternalOutput")

    with TileContext(nc) as tc:
        with (
            tc.tile_pool(name="io", bufs=2) as io,
            tc.tile_pool(name="wk", bufs=1) as wk,
            (tc.For_i(0, repeat, 1) if repeat > 1 else contextlib.nullcontext()),
        ):
            for rt in range(RT):
                r0 = rt * 128
                c0 = 0
                while c0 < OUT_L:
                    C = min(CHUNK, OUT_L - c0)
                    S = C + 4
                    rho_t = io.tile([128, S], F32, tag="rho")
                    nc.sync.dma_start(rho_t[:], rho[r0:r0 + 128, c0:c0 + S])
                    v_t = io.tile([128, S], F32, tag="v")
                    nc.sync.dma_start(v_t[:], vin[r0:r0 + 128, c0:c0 + S])
                    if variant == "dma":
                        out_t = io.tile([128, C], F32, tag="out")
                        nc.vector.tensor_tensor(
                            out_t[:, 0:1], rho_t[:, 0:1], v_t[:, 0:1],
                            AluOpType.mult,
                        )
                        nc.sync.dma_start(
                            out[r0:r0 + 128, c0:c0 + C], out_t[:]
                        )
                        c0 += C
                        continue

                    # ACT: input conversions fp32 -> bf16 (off-DVE).
                    rho_b = wk.tile([128, S], BF16, tag="rho_b")
                    nc.scalar.activation(rho_b[:], rho_t[:], COPY)
                    v_b = wk.tile([128, S], BF16, tag="v_b")
                    nc.scalar.activation(v_b[:], v_t[:], COPY)

                    # DVE bf16 pipeline (all TT at 2x, TS at 4x).
                    # 1. f = rho*v
                    f = wk.tile([128, S], BF16, tag="f")
                    nc.vector.tensor_tensor(
                        f[:], rho_b[:], v_b[:], AluOpType.mult
                    )
                    # 2. d[i] = f[i+1]-f[i]   (S-1 = C+3 cols)
                    d = wk.tile([128, S - 1], BF16, tag="d")
                    nc.vector.tensor_tensor(
                        d[:], f[:, 1:S], f[:, 0:S - 1], AluOpType.subtract
                    )
                    # 3./4. u = min(d0,d1), w = max(d0,d1)   (C+2 cols)
                    u = wk.tile([128, C + 2], BF16, tag="u")
                    nc.vector.tensor_tensor(
                        u[:], d[:, 0:C + 2], d[:, 1:C + 3], AluOpType.min
                    )
                    w = wk.tile([128, C + 2], BF16, tag="w")
                    nc.vector.tensor_tensor(
                        w[:], d[:, 0:C + 2], d[:, 1:C + 3], AluOpType.max
                    )
                    # 5. s = d0+d1 = f[i+2]-f[i];  s4 = 0.25*s (TS, 4x)
                    s = wk.tile([128, C + 2], BF16, tag="s")
                    nc.vector.tensor_tensor(
                        s[:], f[:, 2:C + 4], f[:, 0:C + 2], AluOpType.subtract
                    )
                    nc.vector.tensor_scalar(
                        s[:], s[:], 0.25, None, AluOpType.mult
                    )
                    # 6./7. lo = min(u, s4) -> u;  hi = max(w, s4) -> w
                    nc.vector.tensor_tensor(
                        u[:], u[:], s[:], AluOpType.min
                    )
                    nc.vector.tensor_tensor(
                        w[:], w[:], s[:], AluOpType.max
                    )
                    # 8. hi0 = min(hi, 0) -> s (TS, 4x)
                    nc.vector.tensor_scalar(
                        s[:], w[:], 0.0, None, AluOpType.min
                    )
                    # 9. hs = max(lo, hi0) -> d[:, 0:C+2] (d is dead)
                    hs = d
                    nc.vector.tensor_tensor(
                        hs[:, 0:C + 2], u[:], s[:], AluOpType.max
                    )
                    # 10./11. p = f1 - hs;  q = f1 + hs
                    p = wk.tile([128, C + 2], BF16, tag="p")
                    nc.vector.tensor_tensor(
                        p[:], f[:, 1:C + 3], hs[:, 0:C + 2], AluOpType.subtract
                    )
                    q = wk.tile([128, C + 2], BF16, tag="q")
                    nc.vector.tensor_tensor(
                        q[:], f[:, 1:C + 3], hs[:, 0:C + 2], AluOpType.add
                    )
                    # 12./13. masks from v_b (TS, 4x): mneg=(v1<0), mpos=(v1>0)
                    # written into dead buffers rho_b / f.
                    mneg = rho_b
                    nc.vector.tensor_scalar(
                        mneg[:, 0:C + 2], v_b[:, 1:C + 3], 0.0, None,
                        AluOpType.is_lt,
                    )
                    mpos = f
                    nc.vector.tensor_scalar(
                        mpos[:, 0:C + 2], v_b[:, 1:C + 3], 0.0, None,
                        AluOpType.is_gt,
                    )
                    # 14./15. pm = p*mneg -> p;  qm = q*mpos -> q
                    nc.vector.tensor_tensor(
                        p[:], p[:], mneg[:, 0:C + 2], AluOpType.mult
                    )
                    nc.vector.tensor_tensor(
                        q[:], q[:], mpos[:, 0:C + 2], AluOpType.mult
                    )
                    # global boundary conditions
                    if c0 == 0:
                        nc.vector.memset(q[:, 0:1], 0.0)
                    if c0 + C == OUT_L:
                        nc.vector.memset(p[:, C + 1:C + 2], 0.0)
                    # 16. fn = pm[1:] + qm[:-1] -> u[:, 0:C+1] (u is dead)
                    fn = u
                    nc.vector.tensor_tensor(
                        fn[:, 0:C + 1], p[:, 1:C + 2], q[:, 0:C + 1],
                        AluOpType.add,
                    )
                    # 17. out_b = fn[:-1] - fn[1:] -> w[:, 0:C] (w is dead)
                    out_b = w
                    nc.vector.tensor_tensor(
                        out_b[:, 0:C], fn[:, 0:C], fn[:, 1:C + 1],
                        AluOpType.subtract,
                    )
                    # ACT: final convert bf16 -> fp32 (off-DVE).
                    out_t = io.tile([128, C], F32, tag="out")
                    nc.scalar.activation(out_t[:], out_b[:, 0:C], COPY)
                    nc.sync.dma_start(out[r0:r0 + 128, c0:c0 + C], out_t[:])
                    c0 += C
    _split_multi_waits(nc)
    return nc


_NC_CACHE = None


def _get_nc():
    global _NC_CACHE
    if _NC_CACHE is None:
        _NC_CACHE = build_module()
    return _NC_CACHE


def kernel(rho, v, axis=2, retain_padding=0, **_kw):
    rho = np.ascontiguousarray(np.asarray(rho, dtype=np.float32))
    v = np.ascontiguousarray(np.asarray(v, dtype=np.float32))
    assert rho.shape == (B, M, L) and v.shape == (B, M, L)

    nc = _get_nc()
    in_maps = [
        {
            "rho": rho[c * BP:(c + 1) * BP].reshape(ROWS, L),
            "v": v[c * BP:(c + 1) * BP].reshape(ROWS, L),
        }
        for c in range(NCORES)
    ]
    last_err = None
    for _attempt in range(3):
        try:
            res = run_bass_kernel_spmd(
                nc, in_maps, core_ids=list(range(NCORES))
            )
            break
        except Exception as e:  # rare transient NRT device errors
            last_err = e
            import time as _time
            _time.sleep(5)
    else:
        raise last_err
    outs = [r["out"].reshape(BP, M, OUT_L) for r in res.results]
    return np.concatenate(outs, axis=0)
